# revision 26
# baseline (speedup 1.0000x reference)
"""Low-rank ray tracer CSI kernel for 8 Trainium2 NeuronCores (v6).

Reference computation:
    A = einsum('dpr,kr->dk', ua, F); B = einsum('dpr,kr->dk', ub, F)
    csi[k] = sum_d A[d,k]*B[d,k] / D

Math: with Ua[d,r] = sum_p ua[d,p,r] (same for ub),
    csi[k] = (1/D) f_k^T (Ua^T Ub) f_k = (1/D) f_k^T M f_k
so each core computes its d-shard's p-sums Sa/Sb [DC,R], the tiny Gram
M = Sa^T Sb [R,R], then csi = rowdot(F M^T F^T) -- all on device; the host
just sums the 8 partial [128, K/128] csi tiles.

Perf design, learned from v2-v5 traces:
  * DMA is bound by *landed* SBUF bytes (~426 GB/s fabric), serviced in
    dispatch order; engine instruction streams are FIFO.  All big loads
    ride the sync HWDGE ring in consumption order (int8 DVE chunks early,
    a small one last so the stream's tail consumer is cheap); the scalar
    ring carries only tiny loads and then PSUM drains; scatter DMAs ride
    the gpsimd ring.
  * ub is int8: r[0:56] on the DVE (halving adds: int8 L1 at 1x, fp16
    above -- integer sums <= 2048 are exact in fp16), r[56:64] via
    gpsimd halving adds + a short DVE reduce.
  * ua ships as bf16 ints for the PE: moving operand of a ones-vector
    matmul, p on partitions, two p-halves accumulated into 4-bank
    [1, 2048] PSUM groups; one fp16 scalar drain per group (the last
    group is split scalar/vector to halve its tail latency), gpsimd-ring
    DMAs scatter [1, N] -> [d, r] early, per group, with per-chunk
    dequantization.
  * A burst of dependency-free warm-up matmuls keeps the PE busy through
    the HAM activity window so real matmuls run at 2.4 GHz.
  * Tail: Gram on the PE; per 128-subcarrier chunk, t = ft_chunk^T @ M
    on the PE and a fused DVE tensor_tensor_reduce (t * F -> sum over r)
    writes csi[p, c] directly; one [128, K/128] fp32 store.
"""

import sys

import numpy as np

sys.path.insert(0, "/opt/trn_rl_repo")

import ml_dtypes

import concourse.bacc as bacc
import concourse.bass as bass
import concourse.mybir as mybir
from concourse.bass_utils import run_bass_kernel_spmd
from concourse.tile import TileContext

D, P, R, K = 1024, 256, 64, 1024
NCORES = 8
DC = D // NCORES  # 128 directions per core
PH = P // 2  # 128: p-half on partitions
KC = K // 128  # 8 subcarrier chunks
RC = 14  # r per DVE chunk
NCH_B = 4  # DVE chunks
R_DVE = RC * NCH_B  # 56: r-slice of ub on the DVE tree
R_GPS = R - R_DVE  # 8: r-slice of ub via gpsimd adds
DCH = 32  # d per ua PE chunk
NCH_A = DC // DCH  # 4 ua chunks
QCOLS = DCH * R  # 2048 (d r) columns per PSUM drain group (4 banks)
N_WARM = 20  # PE warm-up matmuls

F32 = mybir.dt.float32
FP16 = mybir.dt.float16
BF16 = mybir.dt.bfloat16
I8 = mybir.dt.int8


def build_bass() -> bass.Bass:
    nc = bacc.Bacc(None, target_bir_lowering=False)
    # PE-path tensor as bf16 ints [P, d, r]; DVE/GPS tensors int8 [d, r, P]
    ua = nc.declare_dram_parameter("ua", [P, DC, R], BF16, isOutput=False)
    ubq = nc.declare_dram_parameter("ubq", [DC, R, P], I8, isOutput=False)
    sab = nc.declare_dram_parameter("sab", [DC, 2 * R], F32, isOutput=False)
    ft = nc.declare_dram_parameter("ft", [R, K], BF16, isOutput=False)
    fn = nc.declare_dram_parameter("fn", [128, KC, R], BF16, isOutput=False)
    out = nc.declare_dram_parameter("out", [128, KC], F32, isOutput=True)

    with TileContext(nc) as tc:
        with (
            nc.allow_low_precision(reason="int8 sums <=2048 are exact in fp16"),
            tc.tile_pool(name="const", bufs=1) as cpool,
            tc.tile_pool(name="achunks", bufs=NCH_A) as apool,
            tc.tile_pool(name="bchunks", bufs=NCH_B) as bpool,
            tc.tile_pool(name="tree", bufs=2) as tpool,
            tc.tile_pool(name="small", bufs=1) as spool,
        ):
            ones = cpool.tile([PH, 1], BF16)
            nc.vector.memset(ones[:], 1.0)
            warm_src = cpool.tile([PH, 512], BF16)
            nc.vector.memset(warm_src[:], 0.0)

            # --- loads ---------------------------------------------------
            # tiny loads on the scalar ring (free until the first drain)
            sab_sb = cpool.tile([DC, 2 * R], F32)
            nc.scalar.dma_start(out=sab_sb[:], in_=sab[:])
            ft_sb = cpool.tile([R, K], BF16)
            nc.scalar.dma_start(out=ft_sb[:], in_=ft[:])
            fn_sb = cpool.tile([128, KC, R], BF16)
            nc.scalar.dma_start(out=fn_sb[:], in_=fn[:])
            # big loads on the sync ring, in consumption order; the int8
            # DVE/GPS loads are batched (two trees per DMA) for better
            # small-transfer efficiency, and lead so the tree never starves
            ua_v = ua.rearrange("(p2 p1) d r -> p1 p2 (d r)", p1=PH)
            bq01 = bpool.tile([DC, 2 * RC, P], I8)
            bq23 = bpool.tile([DC, 2 * RC, P], I8)
            gch = spool.tile([DC, R_GPS, P], I8)
            a_tiles = [
                apool.tile([PH, 2, QCOLS], BF16, tag="ach", name=f"ach{_i}")
                for _i in range(NCH_A)
            ]
            b_tiles = [
                bq01[:, :RC, :], bq01[:, RC:, :], bq23[:, :RC, :], bq23[:, RC:, :]
            ]
            nc.sync.dma_start(out=bq01[:], in_=ubq[:, : 2 * RC, :])
            nc.sync.dma_start(out=a_tiles[0][:], in_=ua_v[:, :, :QCOLS])
            nc.sync.dma_start(out=bq23[:], in_=ubq[:, 2 * RC : 4 * RC, :])
            nc.sync.dma_start(out=gch[:], in_=ubq[:, R_DVE:, :])
            nc.sync.dma_start(out=a_tiles[1][:], in_=ua_v[:, :, QCOLS : 2 * QCOLS])
            nc.sync.dma_start(out=a_tiles[2][:], in_=ua_v[:, :, 2 * QCOLS : 3 * QCOLS])
            nc.sync.dma_start(out=a_tiles[3][:], in_=ua_v[:, :, 3 * QCOLS :])

            stage_a = spool.tile([1, DC * R], FP16)
            saq = spool.tile([DC, R], FP16)
            sbq = spool.tile([DC, R], FP16)
            sa_f = spool.tile([DC, R], BF16)
            sb_f = spool.tile([DC, R], BF16)

            # gpsimd halving adds for ub r[56:64] + short DVE reduce
            g1 = spool.tile([DC, R_GPS, P // 2], FP16)
            nc.gpsimd.tensor_add(
                out=g1[:], in0=gch[:, :, : P // 2], in1=gch[:, :, P // 2 :]
            )
            g2 = spool.tile([DC, R_GPS, P // 4], FP16)
            nc.gpsimd.tensor_add(
                out=g2[:], in0=g1[:, :, : P // 4], in1=g1[:, :, P // 4 :]
            )
            g3 = spool.tile([DC, R_GPS, P // 8], FP16)
            nc.gpsimd.tensor_add(
                out=g3[:], in0=g2[:, :, : P // 8], in1=g2[:, :, P // 8 :]
            )
            nc.vector.tensor_reduce(
                out=sbq[:, R_DVE:],
                in_=g3[:],
                axis=mybir.AxisListType.X,
                op=mybir.AluOpType.add,
            )

            with tc.tile_pool(name="psum_reg", bufs=2, space="PSUM") as rpool:
                # PE warm-up: keep the PE busy through the HAM window
                warm = rpool.tile([1, QCOLS], F32, tag="grp")
                for _ in range(N_WARM):
                    nc.tensor.matmul(
                        warm[:, :512], ones[:], warm_src[:], start=True, stop=True
                    )

                # DVE tree on ub r[0:56]
                for i, bch in enumerate(b_tiles):
                    t1 = tpool.tile([DC, RC, P // 2], FP16, tag="t1")
                    nc.vector.tensor_add(
                        out=t1[:], in0=bch[:, :, : P // 2], in1=bch[:, :, P // 2 :]
                    )
                    t2 = tpool.tile([DC, RC, P // 4], FP16, tag="t2")
                    nc.vector.tensor_add(
                        out=t2[:], in0=t1[:, :, : P // 4], in1=t1[:, :, P // 4 :]
                    )
                    t3 = tpool.tile([DC, RC, P // 8], FP16, tag="t3")
                    nc.vector.tensor_add(
                        out=t3[:], in0=t2[:, :, : P // 8], in1=t2[:, :, P // 8 :]
                    )
                    nc.vector.tensor_reduce(
                        out=sbq[:, i * RC : (i + 1) * RC],
                        in_=t3[:],
                        axis=mybir.AxisListType.X,
                        op=mybir.AluOpType.add,
                    )

                # PE ones-matmul p-sum for ua; drain + scatter + dequant
                # per 2048-column group (last group split scalar/vector)
                for ci, ach in enumerate(a_tiles):
                    grp = rpool.tile([1, QCOLS], F32, tag="grp")
                    for q in range(QCOLS // 512):
                        for p2 in range(2):
                            nc.tensor.matmul(
                                grp[:, q * 512 : (q + 1) * 512],
                                ones[:],
                                ach[:, p2, q * 512 : (q + 1) * 512],
                                start=(p2 == 0),
                                stop=(p2 == 1),
                            )
                    # drain (the last group splits across scalar+vector to
                    # halve its tail latency; each half scatters as soon as
                    # it lands), then one 32-aligned dequantize
                    sl = slice(ci * QCOLS, (ci + 1) * QCOLS)
                    dsl = slice(ci * DCH, (ci + 1) * DCH)
                    if ci == NCH_A - 1:
                        half = QCOLS // 2
                        nd = DCH // 2
                        nc.scalar.copy(
                            out=stage_a[:, sl][:, :half], in_=grp[:, :half]
                        )
                        nc.gpsimd.dma_start(
                            out=saq[ci * DCH : ci * DCH + nd],
                            in_=stage_a[:, sl][:, :half],
                        )
                        nc.scalar.copy(
                            out=stage_a[:, sl][:, half:], in_=grp[:, half:]
                        )
                        nc.gpsimd.dma_start(
                            out=saq[ci * DCH + nd : (ci + 1) * DCH],
                            in_=stage_a[:, sl][:, half:],
                        )
                    else:
                        nc.scalar.copy(out=stage_a[:, sl], in_=grp[:])
                        nc.gpsimd.dma_start(out=saq[dsl], in_=stage_a[:, sl])
                    nc.vector.tensor_mul(
                        out=sa_f[dsl], in0=saq[dsl], in1=sab_sb[dsl, :R]
                    )

                nc.vector.tensor_mul(out=sb_f[:], in0=sbq[:], in1=sab_sb[:, R:])

            with tc.tile_pool(name="psum_tail", bufs=1, space="PSUM") as plt:
                # Gram M[r1,r2] = sum_d Sa[d,r1] Sb[d,r2], scaled by 1/D
                m_psum = plt.tile([R, R], F32, tag="gram")
                nc.tensor.matmul(m_psum[:], sa_f[:], sb_f[:], start=True, stop=True)
                m_sb = spool.tile([R, R], BF16)
                nc.scalar.mul(m_sb[:], m_psum[:], 1.0 / D)

                # per k-chunk: t[k,r2] = sum_r1 ft[r1,k] M[r1,r2] on the PE,
                # then csi[k] = sum_r2 t[k,r2] F[k,r2]: multiply on the DVE
                # and a short segmented reduce
                prod = spool.tile([128, KC, R], BF16)
                csi = spool.tile([128, KC], F32)
                for c in range(KC):
                    t_ps = plt.tile([128, R], F32, tag="t", bufs=4)
                    nc.tensor.matmul(
                        t_ps[:],
                        ft_sb[:, c * 128 : (c + 1) * 128],
                        m_sb[:],
                        start=True,
                        stop=True,
                    )
                    nc.vector.tensor_mul(
                        out=prod[:, c, :], in0=t_ps[:], in1=fn_sb[:, c, :]
                    )
                    nc.vector.tensor_reduce(
                        out=csi[:, c : c + 1],
                        in_=prod[:, c, :],
                        axis=mybir.AxisListType.X,
                        op=mybir.AluOpType.add,
                    )
                    if c == KC // 2 - 1:
                        nc.sync.dma_start(
                            out=out[:, : KC // 2], in_=csi[:, : KC // 2]
                        )
                nc.sync.dma_start(out=out[:, KC // 2 :], in_=csi[:, KC // 2 :])
    nc.compile()
    return nc


def _quant8(x):
    """Per-(d,r) symmetric int8 quantization of [D, P, R] fp32."""
    s = np.abs(x).max(axis=1) / 127.0 + 1e-30  # [D, R]
    q = np.rint(x / s[:, None, :]).astype(np.int8)
    return q, s.astype(np.float32)


def make_in_maps(inputs: dict) -> list[dict]:
    ua = np.asarray(inputs["attenuation_vectors"], dtype=np.float32)
    ub = np.asarray(inputs["radiation_vectors"], dtype=np.float32)
    f = np.asarray(inputs["frequency_basis_vectors"], dtype=np.float32)

    ft = np.ascontiguousarray(f.T.astype(ml_dtypes.bfloat16))  # [R, K]
    # fn[p, c, r] = F[c*128 + p, r]
    fn = np.ascontiguousarray(
        f.reshape(KC, 128, R).transpose(1, 0, 2).astype(ml_dtypes.bfloat16)
    )

    qa, sa = _quant8(ua)
    qb, sb = _quant8(ub)
    sab = np.concatenate([sa, sb], axis=1)  # [D, 2R]
    # PE path ships the quantized integers as bf16 (exact for |q|<=127)
    ua_pe = np.ascontiguousarray(qa.transpose(1, 0, 2).astype(ml_dtypes.bfloat16))
    ub_q = np.ascontiguousarray(qb.transpose(0, 2, 1))  # [D, R, P]

    maps = []
    for c in range(NCORES):
        dsl = slice(c * DC, (c + 1) * DC)
        maps.append(
            {
                "ua": np.ascontiguousarray(ua_pe[:, dsl, :]),
                "ubq": np.ascontiguousarray(ub_q[dsl]),
                "sab": np.ascontiguousarray(sab[dsl]),
                "ft": ft,
                "fn": fn,
            }
        )
    return maps


_NC_CACHE = None


def kernel(**inputs: np.ndarray) -> np.ndarray:
    global _NC_CACHE
    if _NC_CACHE is None:
        _NC_CACHE = build_bass()
    nc = _NC_CACHE

    in_maps = make_in_maps(inputs)
    res = run_bass_kernel_spmd(nc, in_maps, list(range(NCORES)))
    acc = np.zeros((128, KC), dtype=np.float32)
    for r in res.results:
        acc += r["out"]
    return acc.T.reshape(K).astype(np.float32)


if __name__ == "__main__":
    rng = np.random.default_rng(0)
    ins = {
        "attenuation_vectors": rng.standard_normal((D, P, R), dtype=np.float32),
        "radiation_vectors": rng.standard_normal((D, P, R), dtype=np.float32),
        "frequency_basis_vectors": rng.standard_normal((K, R), dtype=np.float32),
    }
    got = kernel(**ins)
    ua_s = ins["attenuation_vectors"].sum(axis=1)
    ub_s = ins["radiation_vectors"].sum(axis=1)
    a = ua_s @ ins["frequency_basis_vectors"].T
    b = ub_s @ ins["frequency_basis_vectors"].T
    want = (a * b).sum(axis=0) / D
    err = np.abs(got - want).max() / np.abs(want).max()
    print("rel err vs local numpy:", err)


# revision 27
# speedup vs baseline: 1.0056x; 1.0056x over previous
"""Low-rank ray tracer CSI kernel for 8 Trainium2 NeuronCores (v6).

Reference computation:
    A = einsum('dpr,kr->dk', ua, F); B = einsum('dpr,kr->dk', ub, F)
    csi[k] = sum_d A[d,k]*B[d,k] / D

Math: with Ua[d,r] = sum_p ua[d,p,r] (same for ub),
    csi[k] = (1/D) f_k^T (Ua^T Ub) f_k = (1/D) f_k^T M f_k
so each core computes its d-shard's p-sums Sa/Sb [DC,R], the tiny Gram
M = Sa^T Sb [R,R], then csi = rowdot(F M^T F^T) -- all on device; the host
just sums the 8 partial [128, K/128] csi tiles.

Perf design, learned from v2-v5 traces:
  * DMA is bound by *landed* SBUF bytes (~426 GB/s fabric), serviced in
    dispatch order; engine instruction streams are FIFO.  All big loads
    ride the sync HWDGE ring in consumption order (int8 DVE chunks early,
    a small one last so the stream's tail consumer is cheap); the scalar
    ring carries only tiny loads and then PSUM drains; scatter DMAs ride
    the gpsimd ring.
  * ub is int8: r[0:56] on the DVE (halving adds: int8 L1 at 1x, fp16
    above -- integer sums <= 2048 are exact in fp16), r[56:64] via
    gpsimd halving adds + a short DVE reduce.
  * ua ships as bf16 ints for the PE: moving operand of a ones-vector
    matmul, p on partitions, two p-halves accumulated into 4-bank
    [1, 2048] PSUM groups; one fp16 scalar drain per group (the last
    group is split scalar/vector to halve its tail latency), gpsimd-ring
    DMAs scatter [1, N] -> [d, r] early, per group, with per-chunk
    dequantization.
  * A burst of dependency-free warm-up matmuls keeps the PE busy through
    the HAM activity window so real matmuls run at 2.4 GHz.
  * Tail: Gram on the PE; per 128-subcarrier chunk, t = ft_chunk^T @ M
    on the PE and a fused DVE tensor_tensor_reduce (t * F -> sum over r)
    writes csi[p, c] directly; one [128, K/128] fp32 store.
"""

import sys

import numpy as np

sys.path.insert(0, "/opt/trn_rl_repo")

import ml_dtypes

import concourse.bacc as bacc
import concourse.bass as bass
import concourse.mybir as mybir
from concourse.bass_utils import run_bass_kernel_spmd
from concourse.tile import TileContext

D, P, R, K = 1024, 256, 64, 1024
NCORES = 8
DC = D // NCORES  # 128 directions per core
PH = P // 2  # 128: p-half on partitions
KC = K // 128  # 8 subcarrier chunks
RC = 14  # r per DVE chunk
NCH_B = 4  # DVE chunks
R_DVE = RC * NCH_B  # 56: r-slice of ub on the DVE tree
R_GPS = R - R_DVE  # 8: r-slice of ub via gpsimd adds
DCH = 32  # d per ua PE chunk
NCH_A = DC // DCH  # 4 ua chunks
QCOLS = DCH * R  # 2048 (d r) columns per PSUM drain group (4 banks)
N_WARM = 20  # PE warm-up matmuls

F32 = mybir.dt.float32
FP16 = mybir.dt.float16
BF16 = mybir.dt.bfloat16
I8 = mybir.dt.int8


def build_bass() -> bass.Bass:
    nc = bacc.Bacc(None, target_bir_lowering=False)
    # PE-path tensor as bf16 ints [P, d, r]; DVE/GPS tensors int8 [d, r, P]
    ua = nc.declare_dram_parameter("ua", [P, DC, R], BF16, isOutput=False)
    ubq = nc.declare_dram_parameter("ubq", [DC, R, P], I8, isOutput=False)
    sab = nc.declare_dram_parameter("sab", [DC, 2 * R], F32, isOutput=False)
    ft = nc.declare_dram_parameter("ft", [R, K], BF16, isOutput=False)
    fn = nc.declare_dram_parameter("fn", [128, KC, R], BF16, isOutput=False)
    out = nc.declare_dram_parameter("out", [128, KC], F32, isOutput=True)

    with TileContext(nc) as tc:
        with (
            nc.allow_low_precision(reason="int8 sums <=2048 are exact in fp16"),
            tc.tile_pool(name="const", bufs=1) as cpool,
            tc.tile_pool(name="achunks", bufs=NCH_A) as apool,
            tc.tile_pool(name="bchunks", bufs=NCH_B) as bpool,
            tc.tile_pool(name="tree", bufs=2) as tpool,
            tc.tile_pool(name="small", bufs=1) as spool,
        ):
            ones = cpool.tile([PH, 1], BF16)
            nc.vector.memset(ones[:], 1.0)
            warm_src = cpool.tile([PH, 512], BF16)
            nc.vector.memset(warm_src[:], 0.0)

            # --- loads ---------------------------------------------------
            # tiny loads on the scalar ring (free until the first drain)
            sab_sb = cpool.tile([DC, 2 * R], F32)
            nc.scalar.dma_start(out=sab_sb[:], in_=sab[:])
            ft_sb = cpool.tile([R, K], BF16)
            nc.scalar.dma_start(out=ft_sb[:], in_=ft[:])
            fn_sb = cpool.tile([128, KC, R], BF16)
            nc.scalar.dma_start(out=fn_sb[:], in_=fn[:])
            # big loads on the sync ring, in consumption order; the int8
            # DVE/GPS loads are batched (two trees per DMA) for better
            # small-transfer efficiency, and lead so the tree never starves
            ua_v = ua.rearrange("(p2 p1) d r -> p1 p2 (d r)", p1=PH)
            bq01 = bpool.tile([DC, 2 * RC, P], I8)
            bq23 = bpool.tile([DC, 2 * RC, P], I8)
            gch = spool.tile([DC, R_GPS, P], I8)
            a_tiles = [
                apool.tile([PH, 2, QCOLS], BF16, tag="ach", name=f"ach{_i}")
                for _i in range(NCH_A)
            ]
            b_tiles = [
                bq01[:, :RC, :], bq01[:, RC:, :], bq23[:, :RC, :], bq23[:, RC:, :]
            ]
            nc.sync.dma_start(out=bq01[:], in_=ubq[:, : 2 * RC, :])
            nc.sync.dma_start(out=a_tiles[0][:], in_=ua_v[:, :, :QCOLS])
            nc.sync.dma_start(out=bq23[:], in_=ubq[:, 2 * RC : 4 * RC, :])
            nc.sync.dma_start(out=gch[:], in_=ubq[:, R_DVE:, :])
            nc.sync.dma_start(out=a_tiles[1][:], in_=ua_v[:, :, QCOLS : 2 * QCOLS])
            nc.sync.dma_start(out=a_tiles[2][:], in_=ua_v[:, :, 2 * QCOLS : 3 * QCOLS])
            nc.sync.dma_start(out=a_tiles[3][:], in_=ua_v[:, :, 3 * QCOLS :])

            stage_a = spool.tile([1, DC * R], FP16)
            saq = spool.tile([DC, R], FP16)
            sbq = spool.tile([DC, R], FP16)
            sa_f = spool.tile([DC, R], BF16)
            sb_f = spool.tile([DC, R], BF16)

            # gpsimd halving adds for ub r[56:64] + short DVE reduce
            g1 = spool.tile([DC, R_GPS, P // 2], FP16)
            nc.gpsimd.tensor_add(
                out=g1[:], in0=gch[:, :, : P // 2], in1=gch[:, :, P // 2 :]
            )
            g2 = spool.tile([DC, R_GPS, P // 4], FP16)
            nc.gpsimd.tensor_add(
                out=g2[:], in0=g1[:, :, : P // 4], in1=g1[:, :, P // 4 :]
            )
            g3 = spool.tile([DC, R_GPS, P // 8], FP16)
            nc.gpsimd.tensor_add(
                out=g3[:], in0=g2[:, :, : P // 8], in1=g2[:, :, P // 8 :]
            )
            nc.vector.tensor_reduce(
                out=sbq[:, R_DVE:],
                in_=g3[:],
                axis=mybir.AxisListType.X,
                op=mybir.AluOpType.add,
            )

            with tc.tile_pool(name="psum_reg", bufs=2, space="PSUM") as rpool:
                # PE warm-up: keep the PE busy through the HAM window
                warm = rpool.tile([1, QCOLS], F32, tag="grp")
                for _ in range(N_WARM):
                    nc.tensor.matmul(
                        warm[:, :512], ones[:], warm_src[:], start=True, stop=True
                    )

                # DVE tree on ub r[0:56]
                for i, bch in enumerate(b_tiles):
                    t1 = tpool.tile([DC, RC, P // 2], FP16, tag="t1")
                    nc.vector.tensor_add(
                        out=t1[:], in0=bch[:, :, : P // 2], in1=bch[:, :, P // 2 :]
                    )
                    t2 = tpool.tile([DC, RC, P // 4], FP16, tag="t2")
                    nc.vector.tensor_add(
                        out=t2[:], in0=t1[:, :, : P // 4], in1=t1[:, :, P // 4 :]
                    )
                    t3 = tpool.tile([DC, RC, P // 8], FP16, tag="t3")
                    nc.vector.tensor_add(
                        out=t3[:], in0=t2[:, :, : P // 8], in1=t2[:, :, P // 8 :]
                    )
                    nc.vector.tensor_reduce(
                        out=sbq[:, i * RC : (i + 1) * RC],
                        in_=t3[:],
                        axis=mybir.AxisListType.X,
                        op=mybir.AluOpType.add,
                    )

                # PE ones-matmul p-sum for ua; drain + scatter + dequant
                # per 2048-column group (last group split scalar/vector)
                for ci, ach in enumerate(a_tiles):
                    grp = rpool.tile([1, QCOLS], F32, tag="grp")
                    for q in range(QCOLS // 512):
                        for p2 in range(2):
                            nc.tensor.matmul(
                                grp[:, q * 512 : (q + 1) * 512],
                                ones[:],
                                ach[:, p2, q * 512 : (q + 1) * 512],
                                start=(p2 == 0),
                                stop=(p2 == 1),
                            )
                    # drain + scatter (the last group at quarter granularity
                    # so both pipeline under its matmuls), then one
                    # 32-aligned dequantize per chunk
                    sl = slice(ci * QCOLS, (ci + 1) * QCOLS)
                    dsl = slice(ci * DCH, (ci + 1) * DCH)
                    if ci == NCH_A - 1:
                        qc = QCOLS // 4
                        nd = DCH // 4
                        for h in range(4):
                            ssl = slice(ci * QCOLS + h * qc, ci * QCOLS + (h + 1) * qc)
                            nc.scalar.copy(
                                out=stage_a[:, ssl], in_=grp[:, h * qc : (h + 1) * qc]
                            )
                            d0 = ci * DCH + h * nd
                            nc.gpsimd.dma_start(
                                out=saq[d0 : d0 + nd], in_=stage_a[:, ssl]
                            )
                    else:
                        nc.scalar.copy(out=stage_a[:, sl], in_=grp[:])
                        nc.gpsimd.dma_start(out=saq[dsl], in_=stage_a[:, sl])
                    nc.vector.tensor_mul(
                        out=sa_f[dsl], in0=saq[dsl], in1=sab_sb[dsl, :R]
                    )

                nc.vector.tensor_mul(out=sb_f[:], in0=sbq[:], in1=sab_sb[:, R:])

            with tc.tile_pool(name="psum_tail", bufs=1, space="PSUM") as plt:
                # Gram M[r1,r2] = sum_d Sa[d,r1] Sb[d,r2], scaled by 1/D
                m_psum = plt.tile([R, R], F32, tag="gram")
                nc.tensor.matmul(m_psum[:], sa_f[:], sb_f[:], start=True, stop=True)
                m_sb = spool.tile([R, R], BF16)
                nc.scalar.mul(m_sb[:], m_psum[:], 1.0 / D)

                # per k-chunk: t[k,r2] = sum_r1 ft[r1,k] M[r1,r2] on the PE,
                # then csi[k] = sum_r2 t[k,r2] F[k,r2]: multiply on the DVE
                # and a short segmented reduce
                prod = spool.tile([128, KC, R], BF16)
                csi = spool.tile([128, KC], F32)
                for c in range(KC):
                    t_ps = plt.tile([128, R], F32, tag="t", bufs=4)
                    nc.tensor.matmul(
                        t_ps[:],
                        ft_sb[:, c * 128 : (c + 1) * 128],
                        m_sb[:],
                        start=True,
                        stop=True,
                    )
                    nc.vector.tensor_mul(
                        out=prod[:, c, :], in0=t_ps[:], in1=fn_sb[:, c, :]
                    )
                    nc.vector.tensor_reduce(
                        out=csi[:, c : c + 1],
                        in_=prod[:, c, :],
                        axis=mybir.AxisListType.X,
                        op=mybir.AluOpType.add,
                    )
                    if c == KC // 2 - 1:
                        nc.sync.dma_start(
                            out=out[:, : KC // 2], in_=csi[:, : KC // 2]
                        )
                nc.sync.dma_start(out=out[:, KC // 2 :], in_=csi[:, KC // 2 :])
    nc.compile()
    return nc


def _quant8(x):
    """Per-(d,r) symmetric int8 quantization of [D, P, R] fp32."""
    s = np.abs(x).max(axis=1) / 127.0 + 1e-30  # [D, R]
    q = np.rint(x / s[:, None, :]).astype(np.int8)
    return q, s.astype(np.float32)


def make_in_maps(inputs: dict) -> list[dict]:
    ua = np.asarray(inputs["attenuation_vectors"], dtype=np.float32)
    ub = np.asarray(inputs["radiation_vectors"], dtype=np.float32)
    f = np.asarray(inputs["frequency_basis_vectors"], dtype=np.float32)

    ft = np.ascontiguousarray(f.T.astype(ml_dtypes.bfloat16))  # [R, K]
    # fn[p, c, r] = F[c*128 + p, r]
    fn = np.ascontiguousarray(
        f.reshape(KC, 128, R).transpose(1, 0, 2).astype(ml_dtypes.bfloat16)
    )

    qa, sa = _quant8(ua)
    qb, sb = _quant8(ub)
    sab = np.concatenate([sa, sb], axis=1)  # [D, 2R]
    # PE path ships the quantized integers as bf16 (exact for |q|<=127)
    ua_pe = np.ascontiguousarray(qa.transpose(1, 0, 2).astype(ml_dtypes.bfloat16))
    ub_q = np.ascontiguousarray(qb.transpose(0, 2, 1))  # [D, R, P]

    maps = []
    for c in range(NCORES):
        dsl = slice(c * DC, (c + 1) * DC)
        maps.append(
            {
                "ua": np.ascontiguousarray(ua_pe[:, dsl, :]),
                "ubq": np.ascontiguousarray(ub_q[dsl]),
                "sab": np.ascontiguousarray(sab[dsl]),
                "ft": ft,
                "fn": fn,
            }
        )
    return maps


_NC_CACHE = None


def kernel(**inputs: np.ndarray) -> np.ndarray:
    global _NC_CACHE
    if _NC_CACHE is None:
        _NC_CACHE = build_bass()
    nc = _NC_CACHE

    in_maps = make_in_maps(inputs)
    res = run_bass_kernel_spmd(nc, in_maps, list(range(NCORES)))
    acc = np.zeros((128, KC), dtype=np.float32)
    for r in res.results:
        acc += r["out"]
    return acc.T.reshape(K).astype(np.float32)


if __name__ == "__main__":
    rng = np.random.default_rng(0)
    ins = {
        "attenuation_vectors": rng.standard_normal((D, P, R), dtype=np.float32),
        "radiation_vectors": rng.standard_normal((D, P, R), dtype=np.float32),
        "frequency_basis_vectors": rng.standard_normal((K, R), dtype=np.float32),
    }
    got = kernel(**ins)
    ua_s = ins["attenuation_vectors"].sum(axis=1)
    ub_s = ins["radiation_vectors"].sum(axis=1)
    a = ua_s @ ins["frequency_basis_vectors"].T
    b = ub_s @ ins["frequency_basis_vectors"].T
    want = (a * b).sum(axis=0) / D
    err = np.abs(got - want).max() / np.abs(want).max()
    print("rel err vs local numpy:", err)


# revision 28
# speedup vs baseline: 1.0934x; 1.0873x over previous
"""Low-rank ray tracer CSI kernel for 8 Trainium2 NeuronCores (v6).

Reference computation:
    A = einsum('dpr,kr->dk', ua, F); B = einsum('dpr,kr->dk', ub, F)
    csi[k] = sum_d A[d,k]*B[d,k] / D

Math: with Ua[d,r] = sum_p ua[d,p,r] (same for ub),
    csi[k] = (1/D) f_k^T (Ua^T Ub) f_k = (1/D) f_k^T M f_k
so each core computes its d-shard's p-sums Sa/Sb [DC,R], the tiny Gram
M = Sa^T Sb [R,R], then csi = rowdot(F M^T F^T) -- all on device; the host
just sums the 8 partial [128, K/128] csi tiles.

Perf design, learned from v2-v5 traces:
  * DMA is bound by *landed* SBUF bytes (~426 GB/s fabric), serviced in
    dispatch order; engine instruction streams are FIFO.  All big loads
    ride the sync HWDGE ring in consumption order (int8 DVE chunks early,
    a small one last so the stream's tail consumer is cheap); the scalar
    ring carries only tiny loads and then PSUM drains; scatter DMAs ride
    the gpsimd ring.
  * ub is int8, batched two-trees-per-DMA for small-transfer efficiency:
    r[0:56] on the DVE (halving adds: int8 L1 at 1x, fp16 above --
    integer sums <= 2048 are exact in fp16), r[56:64] via gpsimd halving
    adds + a short DVE reduce.
  * ua ships as bf16 ints for the PE: moving operand of a ones-vector
    matmul, p on partitions, two p-halves accumulated into 4-bank
    [1, 2048] PSUM groups; one fp16 scalar drain per group (the last
    group drains and scatters at quarter granularity so the chain
    pipelines under its matmuls), gpsimd-ring DMAs scatter
    [1, N] -> [d, r] early, per group, with per-chunk dequantization.
  * A burst of dependency-free warm-up matmuls keeps the PE busy through
    the HAM activity window so real matmuls run at 2.4 GHz.
  * Tail: Gram on the PE; per 128-subcarrier chunk, t = ft_chunk^T @ M
    on the PE, a DVE multiply with F and a short segmented reduce write
    csi[p, c]; the [128, K/128] fp32 store goes out in two overlapping
    halves.  (nc.vector.tensor_tensor_reduce would fuse the last two DVE
    ops but crashes the device at runtime -- do not use it.)
"""

import sys

import numpy as np

sys.path.insert(0, "/opt/trn_rl_repo")

import ml_dtypes

import concourse.bacc as bacc
import concourse.bass as bass
import concourse.mybir as mybir
from concourse.bass_utils import run_bass_kernel_spmd
from concourse.tile import TileContext

D, P, R, K = 1024, 256, 64, 1024
NCORES = 8
DC = D // NCORES  # 128 directions per core
PH = P // 2  # 128: p-half on partitions
KC = K // 128  # 8 subcarrier chunks
RC = 14  # r per DVE chunk
NCH_B = 4  # DVE chunks
R_DVE = RC * NCH_B  # 56: r-slice of ub on the DVE tree
R_GPS = R - R_DVE  # 8: r-slice of ub via gpsimd adds
DCH = 32  # d per ua PE chunk
NCH_A = DC // DCH  # 4 ua chunks
QCOLS = DCH * R  # 2048 (d r) columns per PSUM drain group (4 banks)
N_WARM = 20  # PE warm-up matmuls

F32 = mybir.dt.float32
FP16 = mybir.dt.float16
BF16 = mybir.dt.bfloat16
I8 = mybir.dt.int8


def build_bass() -> bass.Bass:
    nc = bacc.Bacc(None, target_bir_lowering=False)
    # PE-path tensor as bf16 ints [P, d, r]; DVE/GPS tensors int8 [d, r, P]
    ua = nc.declare_dram_parameter("ua", [P, DC, R], BF16, isOutput=False)
    ubq = nc.declare_dram_parameter("ubq", [DC, R, P], I8, isOutput=False)
    sab = nc.declare_dram_parameter("sab", [DC, 2 * R], F32, isOutput=False)
    ft = nc.declare_dram_parameter("ft", [R, K], BF16, isOutput=False)
    fn = nc.declare_dram_parameter("fn", [128, KC, R], BF16, isOutput=False)
    out = nc.declare_dram_parameter("out", [128, KC], F32, isOutput=True)

    with TileContext(nc) as tc:
        with (
            nc.allow_low_precision(reason="int8 sums <=2048 are exact in fp16"),
            tc.tile_pool(name="const", bufs=1) as cpool,
            tc.tile_pool(name="achunks", bufs=NCH_A) as apool,
            tc.tile_pool(name="bchunks", bufs=NCH_B) as bpool,
            tc.tile_pool(name="tree", bufs=2) as tpool,
            tc.tile_pool(name="small", bufs=1) as spool,
        ):
            ones = cpool.tile([PH, 1], BF16)
            nc.vector.memset(ones[:], 1.0)
            warm_src = cpool.tile([PH, 512], BF16)
            nc.vector.memset(warm_src[:], 0.0)

            # --- loads ---------------------------------------------------
            # tiny loads on the scalar ring (free until the first drain)
            sab_sb = cpool.tile([DC, 2 * R], F32)
            nc.scalar.dma_start(out=sab_sb[:], in_=sab[:])
            ft_sb = cpool.tile([R, K], BF16)
            nc.scalar.dma_start(out=ft_sb[:], in_=ft[:])
            fn_sb = cpool.tile([128, KC, R], BF16)
            nc.scalar.dma_start(out=fn_sb[:], in_=fn[:])
            # big loads on the sync ring, in consumption order; the int8
            # DVE/GPS loads are batched (two trees per DMA) for better
            # small-transfer efficiency, and lead so the tree never starves
            ua_v = ua.rearrange("(p2 p1) d r -> p1 p2 (d r)", p1=PH)
            bq01 = bpool.tile([DC, 2 * RC, P], I8)
            bq23 = bpool.tile([DC, 2 * RC, P], I8)
            gch = spool.tile([DC, R_GPS, P], I8)
            a_tiles = [
                apool.tile([PH, 2, QCOLS], BF16, tag="ach", name=f"ach{_i}")
                for _i in range(NCH_A)
            ]
            b_tiles = [
                bq01[:, :RC, :], bq01[:, RC:, :], bq23[:, :RC, :], bq23[:, RC:, :]
            ]
            nc.sync.dma_start(out=bq01[:], in_=ubq[:, : 2 * RC, :])
            nc.sync.dma_start(out=a_tiles[0][:], in_=ua_v[:, :, :QCOLS])
            nc.sync.dma_start(out=bq23[:], in_=ubq[:, 2 * RC : 4 * RC, :])
            nc.sync.dma_start(out=gch[:], in_=ubq[:, R_DVE:, :])
            nc.sync.dma_start(out=a_tiles[1][:], in_=ua_v[:, :, QCOLS : 2 * QCOLS])
            nc.sync.dma_start(out=a_tiles[2][:], in_=ua_v[:, :, 2 * QCOLS : 3 * QCOLS])
            nc.sync.dma_start(out=a_tiles[3][:], in_=ua_v[:, :, 3 * QCOLS :])

            stage_a = spool.tile([1, DC * R], FP16)
            saq = spool.tile([DC, R], FP16)
            sbq = spool.tile([DC, R], FP16)
            sa_f = spool.tile([DC, R], BF16)
            sb_f = spool.tile([DC, R], BF16)

            # gpsimd halving adds for ub r[56:64] + short DVE reduce
            g1 = spool.tile([DC, R_GPS, P // 2], FP16)
            nc.gpsimd.tensor_add(
                out=g1[:], in0=gch[:, :, : P // 2], in1=gch[:, :, P // 2 :]
            )
            g2 = spool.tile([DC, R_GPS, P // 4], FP16)
            nc.gpsimd.tensor_add(
                out=g2[:], in0=g1[:, :, : P // 4], in1=g1[:, :, P // 4 :]
            )
            g3 = spool.tile([DC, R_GPS, P // 8], FP16)
            nc.gpsimd.tensor_add(
                out=g3[:], in0=g2[:, :, : P // 8], in1=g2[:, :, P // 8 :]
            )
            nc.vector.tensor_reduce(
                out=sbq[:, R_DVE:],
                in_=g3[:],
                axis=mybir.AxisListType.X,
                op=mybir.AluOpType.add,
            )

            with tc.tile_pool(name="psum_reg", bufs=2, space="PSUM") as rpool:
                # PE warm-up: keep the PE busy through the HAM window
                warm = rpool.tile([1, QCOLS], F32, tag="grp")
                for _ in range(N_WARM):
                    nc.tensor.matmul(
                        warm[:, :512], ones[:], warm_src[:], start=True, stop=True
                    )

                # DVE tree on ub r[0:56]
                for i, bch in enumerate(b_tiles):
                    t1 = tpool.tile([DC, RC, P // 2], FP16, tag="t1")
                    nc.vector.tensor_add(
                        out=t1[:], in0=bch[:, :, : P // 2], in1=bch[:, :, P // 2 :]
                    )
                    t2 = tpool.tile([DC, RC, P // 4], FP16, tag="t2")
                    nc.vector.tensor_add(
                        out=t2[:], in0=t1[:, :, : P // 4], in1=t1[:, :, P // 4 :]
                    )
                    t3 = tpool.tile([DC, RC, P // 8], FP16, tag="t3")
                    nc.vector.tensor_add(
                        out=t3[:], in0=t2[:, :, : P // 8], in1=t2[:, :, P // 8 :]
                    )
                    nc.vector.tensor_reduce(
                        out=sbq[:, i * RC : (i + 1) * RC],
                        in_=t3[:],
                        axis=mybir.AxisListType.X,
                        op=mybir.AluOpType.add,
                    )

                # PE ones-matmul p-sum for ua; drain + scatter + dequant
                # per 2048-column group (last group split scalar/vector)
                for ci, ach in enumerate(a_tiles):
                    grp = rpool.tile([1, QCOLS], F32, tag="grp")
                    for q in range(QCOLS // 512):
                        for p2 in range(2):
                            nc.tensor.matmul(
                                grp[:, q * 512 : (q + 1) * 512],
                                ones[:],
                                ach[:, p2, q * 512 : (q + 1) * 512],
                                start=(p2 == 0),
                                stop=(p2 == 1),
                            )
                    # drain + scatter (the last group at quarter granularity
                    # so both pipeline under its matmuls), then one
                    # 32-aligned dequantize per chunk
                    sl = slice(ci * QCOLS, (ci + 1) * QCOLS)
                    dsl = slice(ci * DCH, (ci + 1) * DCH)
                    if ci == NCH_A - 1:
                        qc = QCOLS // 4
                        nd = DCH // 4
                        for h in range(4):
                            ssl = slice(ci * QCOLS + h * qc, ci * QCOLS + (h + 1) * qc)
                            nc.scalar.copy(
                                out=stage_a[:, ssl], in_=grp[:, h * qc : (h + 1) * qc]
                            )
                            d0 = ci * DCH + h * nd
                            nc.gpsimd.dma_start(
                                out=saq[d0 : d0 + nd], in_=stage_a[:, ssl]
                            )
                    else:
                        nc.scalar.copy(out=stage_a[:, sl], in_=grp[:])
                        nc.gpsimd.dma_start(out=saq[dsl], in_=stage_a[:, sl])
                    nc.vector.tensor_mul(
                        out=sa_f[dsl], in0=saq[dsl], in1=sab_sb[dsl, :R]
                    )

                nc.vector.tensor_mul(out=sb_f[:], in0=sbq[:], in1=sab_sb[:, R:])

            with tc.tile_pool(name="psum_tail", bufs=1, space="PSUM") as plt:
                # Gram M[r1,r2] = sum_d Sa[d,r1] Sb[d,r2], scaled by 1/D
                m_psum = plt.tile([R, R], F32, tag="gram")
                nc.tensor.matmul(m_psum[:], sa_f[:], sb_f[:], start=True, stop=True)
                m_sb = spool.tile([R, R], BF16)
                nc.scalar.mul(m_sb[:], m_psum[:], 1.0 / D)

                # per k-chunk: t[k,r2] = sum_r1 ft[r1,k] M[r1,r2] on the PE,
                # then csi[k] = sum_r2 t[k,r2] F[k,r2]: multiply on the DVE
                # and a short segmented reduce
                prod = spool.tile([128, KC, R], BF16)
                csi = spool.tile([128, KC], F32)
                for c in range(KC):
                    t_ps = plt.tile([128, R], F32, tag="t", bufs=4)
                    nc.tensor.matmul(
                        t_ps[:],
                        ft_sb[:, c * 128 : (c + 1) * 128],
                        m_sb[:],
                        start=True,
                        stop=True,
                    )
                    nc.vector.tensor_mul(
                        out=prod[:, c, :], in0=t_ps[:], in1=fn_sb[:, c, :]
                    )
                    nc.vector.tensor_reduce(
                        out=csi[:, c : c + 1],
                        in_=prod[:, c, :],
                        axis=mybir.AxisListType.X,
                        op=mybir.AluOpType.add,
                    )
                    if c == KC // 2 - 1:
                        nc.sync.dma_start(
                            out=out[:, : KC // 2], in_=csi[:, : KC // 2]
                        )
                nc.sync.dma_start(out=out[:, KC // 2 :], in_=csi[:, KC // 2 :])
    nc.compile()
    return nc


def _quant8(x):
    """Per-(d,r) symmetric int8 quantization of [D, P, R] fp32."""
    s = np.abs(x).max(axis=1) / 127.0 + 1e-30  # [D, R]
    q = np.rint(x / s[:, None, :]).astype(np.int8)
    return q, s.astype(np.float32)


def make_in_maps(inputs: dict) -> list[dict]:
    ua = np.asarray(inputs["attenuation_vectors"], dtype=np.float32)
    ub = np.asarray(inputs["radiation_vectors"], dtype=np.float32)
    f = np.asarray(inputs["frequency_basis_vectors"], dtype=np.float32)

    ft = np.ascontiguousarray(f.T.astype(ml_dtypes.bfloat16))  # [R, K]
    # fn[p, c, r] = F[c*128 + p, r]
    fn = np.ascontiguousarray(
        f.reshape(KC, 128, R).transpose(1, 0, 2).astype(ml_dtypes.bfloat16)
    )

    qa, sa = _quant8(ua)
    qb, sb = _quant8(ub)
    sab = np.concatenate([sa, sb], axis=1)  # [D, 2R]
    # PE path ships the quantized integers as bf16 (exact for |q|<=127)
    ua_pe = np.ascontiguousarray(qa.transpose(1, 0, 2).astype(ml_dtypes.bfloat16))
    ub_q = np.ascontiguousarray(qb.transpose(0, 2, 1))  # [D, R, P]

    maps = []
    for c in range(NCORES):
        dsl = slice(c * DC, (c + 1) * DC)
        maps.append(
            {
                "ua": np.ascontiguousarray(ua_pe[:, dsl, :]),
                "ubq": np.ascontiguousarray(ub_q[dsl]),
                "sab": np.ascontiguousarray(sab[dsl]),
                "ft": ft,
                "fn": fn,
            }
        )
    return maps


_NC_CACHE = None


def kernel(**inputs: np.ndarray) -> np.ndarray:
    global _NC_CACHE
    if _NC_CACHE is None:
        _NC_CACHE = build_bass()
    nc = _NC_CACHE

    in_maps = make_in_maps(inputs)
    res = run_bass_kernel_spmd(nc, in_maps, list(range(NCORES)))
    acc = np.zeros((128, KC), dtype=np.float32)
    for r in res.results:
        acc += r["out"]
    return acc.T.reshape(K).astype(np.float32)


if __name__ == "__main__":
    rng = np.random.default_rng(0)
    ins = {
        "attenuation_vectors": rng.standard_normal((D, P, R), dtype=np.float32),
        "radiation_vectors": rng.standard_normal((D, P, R), dtype=np.float32),
        "frequency_basis_vectors": rng.standard_normal((K, R), dtype=np.float32),
    }
    got = kernel(**ins)
    ua_s = ins["attenuation_vectors"].sum(axis=1)
    ub_s = ins["radiation_vectors"].sum(axis=1)
    a = ua_s @ ins["frequency_basis_vectors"].T
    b = ub_s @ ins["frequency_basis_vectors"].T
    want = (a * b).sum(axis=0) / D
    err = np.abs(got - want).max() / np.abs(want).max()
    print("rel err vs local numpy:", err)
